# revision 3
# baseline (speedup 1.0000x reference)
"""Sparse-attention head kernel for Trainium2, data-parallel over batch on 8 cores.

v5 design (per core, one batch):
  - q and k are projected W-stationary with fp8 DoubleRow directly into
    transposed layouts (qgT [64, 2048], kT [64, T]) from host-interleaved
    fp8 copies of x (and x[keep]); no PE transposes, no DRAM q roundtrip,
    no indirect gathers anywhere.
  - v is projected x-stationary per t-block ([128, 64] + bias + ones col).
  - S runs fp8 DoubleRow (zero-padded second k-tile): 2x column rate.
  - exp on ACT with the 1/sqrt(C) scale folded; boundary-block masks
    multiply e on DVE (j<8 staircase blocks + one shared tril diagonal).
  - PV accumulates out^T into two pinned PSUM tiles [65, 1024] (row 64 =
    softmax denominator via the ones column); PVs are software-pipelined
    behind the next units' S matmuls.
  - epilogue per q-block j is emitted as soon as column j's accumulation is
    complete (staged), so only j6..j15 remain after the last projection.
"""

import math
import os

if "JAX_PLATFORMS" not in os.environ:
    os.environ["JAX_PLATFORMS"] = "axon,cpu"

import numpy as np
import ml_dtypes

B, T, C = 8, 4096, 1024
HS = 64
KQ = T // 2
NCORES = 8
SCALE = float(C) ** -0.5
BF16 = ml_dtypes.bfloat16
FP8NP = ml_dtypes.float8_e4m3

_NT = T // 128   # 32
_NJ = KQ // 128  # 16
_NG = T // 512   # 8 k-projection groups
_NQ = KQ // 512  # 4 q-projection chunks


def _keep_indices(t):
    a = math.ceil(t / 4)
    keep = [t - 1 - x for x in range(a)]
    keep += [t - 1 - math.ceil(3 / a * (x - a) ** 2 + a) for x in range(a, math.ceil(t / 2))]
    return np.array(list(reversed(keep)), dtype=np.int64)


KEEP = _keep_indices(T)

_FULL, _BOUND, _DEAD = 0, 1, 2
_BLOCK_KIND = np.empty((_NT, _NJ), dtype=np.int64)
for _tb in range(_NT):
    for _j in range(_NJ):
        qlo, qhi = KEEP[_j * 128], KEEP[_j * 128 + 127]
        if 128 * _tb + 127 <= qlo:
            _BLOCK_KIND[_tb, _j] = _FULL
        elif 128 * _tb > qhi:
            _BLOCK_KIND[_tb, _j] = _DEAD
        else:
            _BLOCK_KIND[_tb, _j] = _BOUND

_MASK_IDX = {}
for _tb in range(_NT):
    for _j in range(8):
        if _BLOCK_KIND[_tb, _j] == _BOUND:
            _MASK_IDX[(_tb, _j)] = len(_MASK_IDX)
_NMASK = len(_MASK_IDX)
_TRIL_IDX = _NMASK

_JLO = np.empty(_NT, dtype=np.int64)
for _tb in range(_NT):
    _JLO[_tb] = _NJ
    for _j in range(_NJ):
        if _BLOCK_KIND[_tb, _j] != _DEAD:
            _JLO[_tb] = _j
            break

# last alive t-block per q-block j (column j's accumulation completes there)
_JLAST = np.empty(_NJ, dtype=np.int64)
for _j in range(_NJ):
    _JLAST[_j] = (int(KEEP[_j * 128 + 127]) // 128)
# in sweep order [24..31, 0..23], position when column j is final:
# for j>=8 alive t-blocks include 24..j+16 and all of 0..23 -> final at
# sweep end (tb 23); for j<8 final at tb = _JLAST[j] (all <24).


def _host_masks():
    m = np.zeros((128, (_NMASK + 1) * 128), dtype=np.float32)
    for (tb, j), idx in _MASK_IDX.items():
        tvals = 128 * tb + np.arange(128)[:, None]
        kvals = KEEP[j * 128:(j + 1) * 128][None, :]
        m[:, idx * 128:(idx + 1) * 128] = (tvals <= kvals).astype(np.float32)
    r = np.arange(128)[:, None]
    c = np.arange(128)[None, :]
    m[:, _TRIL_IDX * 128:(_TRIL_IDX + 1) * 128] = (r <= c).astype(np.float32)
    return m.astype(BF16)


_prog_cache = {}
TRACE = False
TRACE_KW = {}
LAST_RESULTS = None


def _build_program():
    import concourse.bass as bass  # noqa: F401
    import concourse.mybir as mybir
    import concourse.tile as tile
    from concourse import bacc
    from concourse.masks import make_identity

    dt = mybir.dt
    f32, bf16, fp8 = dt.float32, dt.bfloat16, dt.float8e4
    Alu = mybir.AluOpType
    Act = mybir.ActivationFunctionType
    DR = mybir.MatmulPerfMode.DoubleRow

    nc = bacc.Bacc("TRN2", target_bir_lowering=False, debug=False,
                   enable_partition_id=False)

    xt_d = nc.dram_tensor("xt", [C, T], bf16, kind="ExternalInput").ap()
    # fp8 c-pair k-outer layouts, 512-t/q-group-major:
    # xt8: per group g (512 t): 4096 cols = [gp(4)][two(2)][512]
    xt8_d = nc.dram_tensor("xt8", [128, _NG * 4096], fp8, kind="ExternalInput").ap()
    xq8_d = nc.dram_tensor("xq8", [128, _NQ * 4096], fp8, kind="ExternalInput").ap()
    # w pack for v only (x-stationary): per chunk c, 64 cols of Wv.T rows
    wv_d = nc.dram_tensor("wv", [128, 8 * 64], bf16, kind="ExternalInput").ap()
    # fp8 W for k and q (c-pair k-outer): per gp, 128 cols = [wX(2gp)|wX(2gp+1)]
    wk8_d = nc.dram_tensor("wk8", [128, 4 * 128], fp8, kind="ExternalInput").ap()
    wq8_d = nc.dram_tensor("wq8", [128, 4 * 128], fp8, kind="ExternalInput").ap()
    bv_d = nc.dram_tensor("bv", [1, 64], bf16, kind="ExternalInput").ap()
    bk_d = nc.dram_tensor("bk", [64, 1], f32, kind="ExternalInput").ap()
    bq_d = nc.dram_tensor("bq", [64, 1], f32, kind="ExternalInput").ap()
    masks_d = nc.dram_tensor("masks", [128, (_NMASK + 1) * 128], bf16,
                             kind="ExternalInput").ap()
    out_d = nc.dram_tensor("out", [KQ, HS], f32, kind="ExternalOutput").ap()

    XR = [(3072, 4096), (0, 1024), (1024, 2048), (2048, 3072)]

    with tile.TileContext(nc) as tc:
        with (
            tc.tile_pool(name="const", bufs=1) as constp,
            tc.tile_pool(name="xt", bufs=1) as xtp,
            tc.tile_pool(name="proj", bufs=1) as projp,
            tc.tile_pool(name="psS", bufs=2, space="PSUM") as psS,
            tc.tile_pool(name="psO", bufs=1, space="PSUM") as psO,
            tc.tile_pool(name="work", bufs=2) as workp,
            tc.tile_pool(name="ework", bufs=6) as ep,
        ):
            # constants: DMAs first (gpsimd), identity after
            wv_sb = constp.tile([128, 8 * 64], bf16)
            nc.gpsimd.dma_start(out=wv_sb, in_=wv_d)
            wk8_sb = constp.tile([128, 4 * 128], fp8)
            nc.gpsimd.dma_start(out=wk8_sb, in_=wk8_d)
            wq8_sb = constp.tile([128, 4 * 128], fp8)
            nc.gpsimd.dma_start(out=wq8_sb, in_=wq8_d)
            bv_bc = constp.tile([128, 64], bf16)
            nc.gpsimd.dma_start(out=bv_bc, in_=bv_d.to_broadcast([128, 64]))
            bk_sb = constp.tile([64, 1], f32)
            nc.gpsimd.dma_start(out=bk_sb, in_=bk_d)
            bq_sb = constp.tile([64, 1], f32)
            nc.gpsimd.dma_start(out=bq_sb, in_=bq_d)
            mask_big = constp.tile([128, (_NMASK + 1) * 128], bf16)
            ident_f = constp.tile([128, 128], f32)
            make_identity(nc, ident_f)

            # persistent tensors
            xt_r = [xtp.tile([128, 8 * 1024], bf16, name=f"xt_{i}",
                             tag=f"xt_{i}") for i in range(4)]
            xt8_g = [xtp.tile([128, 4096], fp8, name=f"xt8_{g}",
                              tag=f"xt8_{g}") for g in range(_NG)]
            xq8_g = [xtp.tile([128, 4096], fp8, name=f"xq8_{g}",
                              tag=f"xq8_{g}") for g in range(_NQ)]
            # kT in fp8, per 512-t group: [64, 1024] = [kt(512) | zeros(512)]
            kt_g = [projp.tile([64, 1024], fp8, name=f"kt_{g}",
                               tag=f"kt_{g}") for g in range(_NG)]
            # qgT in fp8: [64, 2*KQ] = [qgT | zeros]
            qgt8 = projp.tile([64, 2 * KQ], fp8)
            nc.vector.memset(qgt8[:, KQ:2 * KQ], 0.0)
            for g in range(_NG):
                nc.vector.memset(kt_g[g][:, 512:1024], 0.0)
            vext_sb = [projp.tile([128, HS + 1], bf16, name=f"vext_{tb}",
                                  tag=f"vext_{tb}") for tb in range(_NT)]
            ps_o_lo = psO.tile([65, 1024], f32, name="po_lo", tag="po_lo")
            ps_o_hi = psO.tile([65, 1024], f32, name="po_hi", tag="po_hi")

            def xt_sl(c, lo, hi):
                ri = [i for i, (a, b) in enumerate(XR) if a <= lo < b][0]
                a = XR[ri][0]
                return xt_r[ri][:, c * 1024 + (lo - a): c * 1024 + (hi - a)]

            # ---- DMA plan ----
            def dma_xt(ri, half=None):
                lo, hi = XR[ri]
                if half is not None:
                    lo, hi = (lo, lo + 512) if half == 0 else (lo + 512, hi)
                for c in range(8):
                    nc.sync.dma_start(
                        out=xt_r[ri][:, c * 1024 + (lo - XR[ri][0]):
                                     c * 1024 + (hi - XR[ri][0])],
                        in_=xt_d[c * 128:(c + 1) * 128, lo:hi])

            def dma_xt8(g):
                nc.gpsimd.dma_start(out=xt8_g[g],
                                    in_=xt8_d[:, g * 4096:(g + 1) * 4096])

            def dma_xq8(g):
                nc.gpsimd.dma_start(out=xq8_g[g],
                                    in_=xq8_d[:, g * 4096:(g + 1) * 4096])

            # prologue needs: xt8 g6,g7 (k for t 3072..4096), xq8 c2,c3
            # (dense-q), xt range0 (v), then the rest.
            dma_xt8(6)
            dma_xt8(7)
            dma_xq8(2)
            dma_xq8(3)
            dma_xt(0, half=0)
            dma_xt(0, half=1)
            dma_xq8(0)
            dma_xq8(1)
            nc.gpsimd.dma_start(out=mask_big, in_=masks_d)
            for g in range(6):
                dma_xt8(g)
            for ri in range(1, 4):
                dma_xt(ri)

            # ---- W-stationary fp8-DR projections (k and q) ----
            def emit_kproj(g):
                ps_k = psS.tile([64, 512], f32, name="ps", tag="ps")
                for gp in range(4):
                    lhs = wk8_sb[:, gp * 128:(gp + 1) * 128].rearrange(
                        "p (two m) -> p two m", two=2)
                    rhs = xt8_g[g][:, gp * 1024:(gp + 1) * 1024].rearrange(
                        "p (two n) -> p two n", two=2)
                    nc.tensor.matmul(ps_k, lhsT=lhs, rhs=rhs,
                                     start=(gp == 0), stop=(gp == 3),
                                     perf_mode=DR)
                nc.vector.tensor_scalar(out=kt_g[g][:, 0:512], in0=ps_k,
                                        scalar1=bk_sb, scalar2=None,
                                        op0=Alu.add)

            def emit_qproj(g):
                ps_q = psS.tile([64, 512], f32, name="ps", tag="ps")
                for gp in range(4):
                    lhs = wq8_sb[:, gp * 128:(gp + 1) * 128].rearrange(
                        "p (two m) -> p two m", two=2)
                    rhs = xq8_g[g][:, gp * 1024:(gp + 1) * 1024].rearrange(
                        "p (two n) -> p two n", two=2)
                    nc.tensor.matmul(ps_q, lhsT=lhs, rhs=rhs,
                                     start=(gp == 0), stop=(gp == 3),
                                     perf_mode=DR)
                nc.vector.tensor_scalar(out=qgt8[:, g * 512:(g + 1) * 512],
                                        in0=ps_q, scalar1=bq_sb, scalar2=None,
                                        op0=Alu.add)

            # ---- v projection per t-block (x-stationary) ----
            def emit_vproj(tb):
                t0 = tb * 128
                ps_v = psS.tile([128, 64], f32, name="ps", tag="ps")
                for c in range(8):
                    nc.tensor.matmul(ps_v, lhsT=xt_sl(c, t0, t0 + 128),
                                     rhs=wv_sb[:, c * 64:(c + 1) * 64],
                                     start=(c == 0), stop=(c == 7))
                nc.vector.tensor_tensor(out=vext_sb[tb][:, 0:HS], in0=ps_v,
                                        in1=bv_bc, op=Alu.add)
                nc.vector.memset(vext_sb[tb][:, HS:HS + 1], 1.0)

            # ---- attention ----
            def s_matmul(ps_slice, tb, q0, q1):
                g, o = tb // 4, (tb % 4) * 128
                lhs = kt_g[g].rearrange("p (two m) -> p two m",
                                        two=2)[:, :, o:o + 128]
                rhs = qgt8.rearrange("p (two n) -> p two n", two=2)[:, :, q0:q1]
                nc.tensor.matmul(ps_slice, lhsT=lhs, rhs=rhs,
                                 start=True, stop=True, perf_mode=DR)

            zrow = constp.tile([1, 512], bf16)
            nc.vector.memset(zrow, 0.0)

            def emit_zero_pso():
                for tile_ in (ps_o_lo, ps_o_hi):
                    for b0 in (0, 512):
                        nc.tensor.matmul(tile_[0:65, b0:b0 + 512],
                                         lhsT=zrow[:, 0:65], rhs=zrow,
                                         start=True, stop=False,
                                         skip_group_check=True)

            pending_pv = []

            def flush_pv(keep=0):
                while len(pending_pv) > keep:
                    tb, e_sl, q0, qm, stop = pending_pv.pop(0)
                    p0 = q0
                    while p0 < qm:
                        pb = min((p0 // 512 + 1) * 512, qm)
                        blk = p0 // 512
                        tgt = ps_o_lo if blk < 2 else ps_o_hi
                        base = 0 if blk < 2 else 1024
                        nc.tensor.matmul(
                            tgt[:, p0 - base:pb - base], lhsT=vext_sb[tb],
                            rhs=e_sl[:, p0 - q0:pb - q0],
                            start=False, stop=stop, skip_group_check=True)
                        p0 = pb

            def mask_block(e_sb, tb, j, o):
                midx = None
                if j < 8 and _BLOCK_KIND[tb, j] == _BOUND:
                    midx = _MASK_IDX[(tb, j)]
                elif j >= 8 and tb == j + 16:
                    midx = _TRIL_IDX
                if midx is not None:
                    nc.vector.tensor_tensor(
                        out=e_sb[:, o:o + 128], in0=e_sb[:, o:o + 128],
                        in1=mask_big[:, midx * 128:(midx + 1) * 128],
                        op=Alu.mult)

            def emit_att(tb, q0, q1, stop=False):
                while q0 < q1:
                    qm = min((q0 // 1024 + 1) * 1024, q1)
                    ps_s = psS.tile([128, 1024], f32, name="ps", tag="ps")
                    for s0 in range(q0, qm, 512):
                        s1 = min(s0 + 512, qm)
                        s_matmul(ps_s[:, s0 - q0:s1 - q0], tb, s0, s1)
                    prev = list(pending_pv)
                    pending_pv.clear()
                    e_sb = ep.tile([128, 1024], bf16, name="e_sb")
                    w = qm - q0
                    nc.scalar.activation(e_sb[:, 0:w], ps_s[:, 0:w],
                                         Act.Exp, scale=SCALE)
                    for j in range(q0 // 128, qm // 128):
                        mask_block(e_sb, tb, j, j * 128 - q0)
                    pending_pv.extend(prev)
                    flush_pv(keep=4)
                    pending_pv.append((tb, e_sb[:, 0:w], q0, qm, stop))
                    q0 = qm

            # ---- column strips for t-blocks 24..31 ----
            def emit_strip_pair(p):
                j0 = 8 + 2 * p
                items = []
                for tb in range(24, 26 + 2 * p):
                    c0 = max(j0 * 128, (tb - 16) * 128)
                    c1 = (j0 + 2) * 128
                    if c0 < c1:
                        items.append((tb, c0, c1))

                def flush_group(g):
                    if not g:
                        return
                    ps_s = psS.tile([128, 1024], f32, name="ps", tag="ps")
                    ofs = 0
                    placed = []
                    for (tb, c0, c1) in g:
                        s_matmul(ps_s[:, ofs:ofs + (c1 - c0)], tb, c0, c1)
                        placed.append((tb, c0, c1, ofs))
                        ofs += c1 - c0
                    prev = list(pending_pv)
                    pending_pv.clear()
                    e_sb = ep.tile([128, 1024], bf16, name="e_sb")
                    nc.scalar.activation(e_sb[:, 0:ofs], ps_s[:, 0:ofs],
                                         Act.Exp, scale=SCALE)
                    for (tb, c0, c1, o) in placed:
                        for j in range(c0 // 128, c1 // 128):
                            mask_block(e_sb, tb, j, o + j * 128 - c0)
                    pending_pv.extend(prev)
                    flush_pv(keep=4)
                    for (tb, c0, c1, o) in placed:
                        pending_pv.append(
                            (tb, e_sb[:, o:o + (c1 - c0)], c0, c1, False))

                g, used = [], 0
                for it in items:
                    wdt = it[2] - it[1]
                    if used + wdt > 1024:
                        flush_group(g)
                        g, used = [], 0
                    g.append(it)
                    used += wdt
                flush_group(g)

            # ---- staged per-j epilogue ----
            out8_lo = workp.tile([128, 8 * HS], f32, name="out8lo", tag="o8lo")
            out8_hi = workp.tile([128, 8 * HS], f32, name="out8hi", tag="o8hi")

            def emit_epi_j(j):
                ps_o = ps_o_lo if j < 8 else ps_o_hi
                base = 0 if j < 8 else 1024
                out8 = out8_lo if j < 8 else out8_hi
                jj = j % 8
                ot = workp.tile([65, 128], f32, name="ot", tag=f"ot{j % 2}")
                nc.vector.tensor_copy(
                    ot, ps_o[:, j * 128 - base:(j + 1) * 128 - base])
                ps_on = psS.tile([128, 65], f32, name="ps", tag="ps")
                nc.tensor.transpose(ps_on, ot, ident_f[0:65, 0:65])
                rec = workp.tile([128, 1], f32, name="rec", tag=f"rec{j % 2}")
                nc.vector.reciprocal(rec, ps_on[:, HS:HS + 1])
                nc.vector.tensor_scalar(
                    out=out8[:, jj * HS:(jj + 1) * HS], in0=ps_on[:, 0:HS],
                    scalar1=rec[:, :1], scalar2=None, op0=Alu.mult)

            def emit_out_dma(which):
                out8 = out8_lo if which == "lo" else out8_hi
                qbase = 0 if which == "lo" else 1024
                out_view = out_d[qbase:qbase + 1024, :].rearrange(
                    "(j p) d -> p j d", p=128)
                nc.sync.dma_start(out=out_view,
                                  in_=out8.rearrange("p (j d) -> p j d", j=8))

            # ---- schedule ----
            emit_zero_pso()
            emit_kproj(6)
            emit_kproj(7)
            emit_qproj(2)
            emit_qproj(3)
            for i, tb in enumerate(range(24, 32)):
                emit_vproj(tb)
                if i == 0:
                    emit_strip_pair(0)
                if i == 2:
                    emit_strip_pair(1)
                if i == 4:
                    emit_strip_pair(2)
                if i == 6:
                    emit_strip_pair(3)
            emit_qproj(0)
            emit_qproj(1)
            emit_kproj(0)
            emit_kproj(1)
            # PH2: v-proj leads attention by 2 t-blocks; k groups 2..5
            # interleave early (group g needed by att from tb 4g).
            next_kg = 2
            epi_q = [j for j in range(8)]  # staged lo epilogues by _JLAST
            for tb in range(0, 24):
                emit_vproj(tb)
                if next_kg < 6 and tb >= 2 * (next_kg - 2):
                    emit_kproj(next_kg)
                    next_kg += 1
                if tb > 1:
                    emit_att(tb - 2, int(_JLO[tb - 2]) * 128, 2048)
                    tau = tb - 2
                    while epi_q and _JLAST[epi_q[0]] <= tau - 3:
                        emit_epi_j(epi_q.pop(0))
            emit_att(22, int(_JLO[22]) * 128, 2048)
            emit_att(23, int(_JLO[23]) * 128, 2048, stop=True)
            flush_pv()
            while epi_q:
                emit_epi_j(epi_q.pop(0))
            emit_out_dma("lo")
            for j in range(8, 16):
                emit_epi_j(j)
            emit_out_dma("hi")

    nc.compile()
    return nc


def _get_program():
    if "nc" not in _prog_cache:
        _prog_cache["nc"] = _build_program()
    return _prog_cache["nc"]


def _host_pack(Wq, bq, Wk, bk, Wv, bv):
    WqT = np.asarray(Wq).T.astype(np.float32)  # [C, 64]
    WkT = np.asarray(Wk).T.astype(np.float32)
    WvT = np.asarray(Wv).T.astype(np.float32)
    wv_pack = np.empty((128, 8 * 64), dtype=np.float32)
    for c in range(8):
        wv_pack[:, c * 64:(c + 1) * 64] = WvT[c * 128:(c + 1) * 128, :]
    def w8(WT):
        o = np.empty((128, 4 * 128), dtype=np.float32)
        for gp in range(4):
            o[:, gp * 128:gp * 128 + 64] = WT[2 * gp * 128:(2 * gp + 1) * 128, :]
            o[:, gp * 128 + 64:(gp + 1) * 128] = WT[(2 * gp + 1) * 128:(2 * gp + 2) * 128, :]
        return o.astype(FP8NP)
    return (wv_pack.astype(BF16), w8(WkT), w8(WqT),
            np.asarray(bv).astype(np.float32)[None, :].astype(BF16),
            np.asarray(bk).astype(np.float32).reshape(HS, 1),
            np.asarray(bq).astype(np.float32).reshape(HS, 1))


def _host_x8(xrows):
    """[N, C] rows -> fp8 [128, (N//512)*4096], 512-row groups of
    [gp(4)][two(2)][512] c-pair k-outer layout."""
    n = xrows.shape[0]
    ng = n // 512
    o = np.empty((128, ng * 4096), dtype=np.float32)
    xT = xrows.T  # [C, N]
    for g in range(ng):
        for gp in range(4):
            base = g * 4096 + gp * 1024
            o[:, base:base + 512] = xT[2 * gp * 128:(2 * gp + 1) * 128,
                                       g * 512:(g + 1) * 512]
            o[:, base + 512:base + 1024] = xT[(2 * gp + 1) * 128:(2 * gp + 2) * 128,
                                              g * 512:(g + 1) * 512]
    return np.ascontiguousarray(o.astype(FP8NP))


def kernel(x, Wq, bq, Wk, bk, Wv, bv):
    from concourse.bass_utils import run_bass_kernel_spmd

    x = np.asarray(x, dtype=np.float32)
    wv_pack, wk8, wq8, bv_r, bk_c, bq_c = _host_pack(Wq, bq, Wk, bk, Wv, bv)
    masks = _host_masks()

    nc = _get_program()
    in_maps = []
    for b in range(NCORES):
        xb = x[b]
        in_maps.append({
            "xt": np.ascontiguousarray(xb.T).astype(BF16),
            "xt8": _host_x8(xb),
            "xq8": _host_x8(xb[KEEP]),
            "wv": wv_pack,
            "wk8": wk8,
            "wq8": wq8,
            "bv": bv_r,
            "bk": bk_c,
            "bq": bq_c,
            "masks": masks,
        })
    res = run_bass_kernel_spmd(nc, in_maps, core_ids=list(range(NCORES)),
                               trace=TRACE, **TRACE_KW)
    global LAST_RESULTS
    LAST_RESULTS = res
    out = np.stack([res.results[b]["out"] for b in range(NCORES)], axis=0)
    return out.astype(np.float32)
